# revision 21
# baseline (speedup 1.0000x reference)
"""CTC batch loss (Keras convention, blank = C-1) on 8 Trainium2 NeuronCores.

Strategy (pure data parallel, 128 examples per core = 128 SBUF partitions):
  * Prob-domain scaled forward DP, identical math to the reference's
    log-space DP including the log(p + 1e-7) epsilon (folded into the
    gather as E = onehot + eps; softmax rows sum to 1 so the gathered
    value is exactly p + eps).
  * Gather via per-example one-hot matmul on the TensorEngine: y_pred
    arrives host-cast to f16 in t-chunk-major layout; XBAR dma-transpose
    (t,c)->(c,t); matmul E^T . X^T -> PSUM f32 [65, t]; ACT-engine escape
    copy casting to bf16; DMA re-layout into batch-partitioned p_store.
  * Serial DP over T in bf16 with a split even/odd contiguous layout so
    most hot ops run in the DVE 2x/4x packed perf modes: 6 DVE ops per
    step, no cross-engine hops; rescale every 8 steps, log-corrections
    collected in a strip and reduced once at the end.
"""

import sys
from contextlib import ExitStack

import numpy as np

for _p in ("/opt/trn_rl_repo",):
    if _p not in sys.path:
        sys.path.insert(0, _p)

import concourse.bass as bass
import concourse.tile as tile
from concourse import mybir
from concourse.bass_utils import run_bass_kernel_spmd

# Problem constants (hardcoded per spec nn_CTC_55808805045003)
B, T, C, L = 1024, 256, 128, 64
NCORES = 8
BL = B // NCORES          # 128 examples per core
S = 2 * L + 1             # 129 extended labels
NS = L + 1                # 65 gather columns (64 labels + blank)
EPS = 1e-7
CH = 128                  # time chunk
NCH = T // CH             # 2
RESC = 8                  # rescale period
GT = 8                    # examples per transpose/matmul-group/relayout DMA

f32 = mybir.dt.float32
bf16 = mybir.dt.bfloat16
f16 = mybir.dt.float16

ADD = mybir.AluOpType.add
MULT = mybir.AluOpType.mult
AX_X = mybir.AxisListType.X
AFT = mybir.ActivationFunctionType


def _body(tc, loss_ap, yp16, e_ap, m_ap, rpb_ap):
    nc = tc.nc
    with ExitStack() as ctx:
        const = ctx.enter_context(tc.tile_pool(name="const", bufs=1))
        xtp = ctx.enter_context(tc.tile_pool(name="xt", bufs=6))
        gps = ctx.enter_context(tc.tile_pool(name="gpsum", bufs=3, space="PSUM"))
        gsb = ctx.enter_context(tc.tile_pool(name="gsb", bufs=6))
        tiny = ctx.enter_context(tc.tile_pool(name="tiny", bufs=8))

        # ---- label-derived constants (host-computed, DMA'd in) ----
        Eall = const.tile([128, BL * NS], f16)
        nc.sync.dma_start(Eall[:], e_ap[:, :])
        m_odd = const.tile([128, 63], bf16)
        nc.sync.dma_start(m_odd[:], m_ap[:, :])

        # per-chunk gathered probs (bf16): p_store[ch][b, s*CH + t]
        # rows 0..63 are divided by the blank row in place (q-transform)
        p_stores = [
            const.tile([128, NS * CH], bf16, name=f"p_store{ch}") for ch in range(NCH)
        ]
        ps3 = [
            p_stores[ch][:].rearrange("p (s t) -> p s t", s=NS) for ch in range(NCH)
        ]
        # strip: 31 rescale logs + 1 final lik col
        strip = const.tile([128, 32], f32)
        # host-computed 1/(p_blank+eps) in bf16, replicated across the 64
        # label rows so the q-divide is one plain packed multiply per chunk
        rpbr = [const.tile([128, 64 * CH], bf16, name=f"rpbr{ch}") for ch in range(NCH)]
        for ch in range(NCH):
            nc.sync.dma_start(rpbr[ch][:], rpb_ap[ch * BL : (ch + 1) * BL, :])

        def gather_chunk(ch):
            for g in range(BL // GT):
                b0 = g * GT
                r0 = ch * BL + b0
                eng_t = nc.sync if (g % 2 == 0) else nc.scalar
                # one XBAR transpose covers GT examples: [GT*CH, C] -> [C, GT*CH]
                xt = xtp.tile([C, GT * CH], f16)
                eng_t.dma_start_transpose(
                    xt[:],
                    yp16[r0 : r0 + GT, :, :].rearrange("b t c -> (b t) c"),
                )
                gp = gps.tile([NS, GT * CH], f32)
                for i in range(GT):
                    b = b0 + i
                    # out[s, t] = sum_c E[c, s] * xt[c, t]
                    nc.tensor.matmul(
                        gp[:, i * CH : (i + 1) * CH],
                        Eall[:, b * NS : (b + 1) * NS],
                        xt[:, i * CH : (i + 1) * CH],
                        start=True,
                        stop=True,
                    )
                # escape PSUM f32 -> SBUF bf16 (GpSimd can't read PSUM):
                # DVE pre-DP for chunk 0, ACT during DP for chunk 1
                gs = gsb.tile([NS, GT * CH], bf16)
                if ch == 0:
                    nc.vector.tensor_copy(gs[:], gp[:])
                else:
                    nc.scalar.copy(gs[:], gp[:])
                # per-example re-layout into batch partitions; chunk 0 uses
                # three HWDGE queues (DVE's is free pre-DP), chunk 1 two
                for i in range(GT):
                    b = b0 + i
                    if ch == 0:
                        r = b % 16
                        eng = nc.sync if r < 5 else (nc.scalar if r < 10 else nc.gpsimd)
                    else:
                        eng = nc.scalar if (b % 2) else nc.sync
                    eng.dma_start(
                        ps3[ch][b : b + 1, :, :], gs[:, i * CH : (i + 1) * CH]
                    )

        def q_transform(ch, mult_engine):
            """Divide label rows by the host-computed blank reciprocal."""
            rv = rpbr[ch][:].rearrange("p (s t) -> p s t", s=64)
            mult_engine.tensor_tensor(
                ps3[ch][:, 0:64, :], ps3[ch][:, 0:64, :], rv, MULT
            )

        gather_chunk(0)
        q_transform(0, nc.vector)
        gather_chunk(1)

        # ---- DP state (bf16, split even/odd contiguous layout) ----
        # alpha cols: 0 = zero guard; 1..65 = even states i=0..64 (s=2i);
        # 66 = zero guard; 67..130 = odd states j=0..63 (s=2j+1); 131 pad
        alpha = const.tile([128, 132], bf16)
        AM = const.tile([128, 64], bf16)   # AM[k] = mask*alpha_odd[k-1]; AM[0]=0
        uo = const.tile([128, 64], bf16)
        vo = const.tile([128, 64], bf16)
        r_ap = tiny.tile([128, 1], f32)

        nc.vector.memset(alpha[:], 0.0)
        nc.vector.memset(AM[:], 0.0)

        # t = 0 init (q-transformed): alpha[s=0] = 1, alpha[s=1] = q_lab0(0),
        # AM[1] = m0 * alpha_odd0
        nc.vector.memset(alpha[:, 1:2], 1.0)
        nc.vector.tensor_copy(alpha[:, 67:68], ps3[0][:, 0:1, 0:1].squeeze(2))
        nc.vector.tensor_tensor(AM[:, 1:2], alpha[:, 67:68], m_odd[:, 0:1], MULT)

        V = nc.vector
        k_resc = 0
        for t in range(1, T):
            ch, tt = divmod(t, CH)
            ps = ps3[ch]
            ql = ps[:, 0:64, tt : tt + 1].squeeze(2)  # [128, 64] q = p_lab/p_bl
            # uo[j] = alpha_odd[j] + alpha_even[j]
            V.tensor_tensor(uo[:], alpha[:, 67:131], alpha[:, 1:65], ADD)
            # vo[j] = uo[j] + AM[j]
            V.tensor_tensor(vo[:], uo[:], AM[:], ADD)
            # even' = even + odd shifted (no multiply in q-space)
            V.tensor_tensor(alpha[:, 1:66], alpha[:, 1:66], alpha[:, 66:131], ADD)
            # odd' = vo * q_lab
            V.tensor_tensor(alpha[:, 67:131], vo[:], ql, MULT)
            # AM' for next step: mask * alpha_odd' shifted
            V.tensor_tensor(AM[:, 1:64], alpha[:, 67:130], m_odd[:], MULT)
            if t % RESC == 0 and t < T - RESC + 1:
                V.tensor_reduce(strip[:, k_resc : k_resc + 1], alpha[:, 1:131], AX_X, ADD)
                V.reciprocal(r_ap[:], strip[:, k_resc : k_resc + 1])
                V.tensor_scalar(alpha[:, 1:131], alpha[:, 1:131], r_ap[:], None, MULT)
                V.tensor_scalar(AM[:], AM[:], r_ap[:], None, MULT)
                k_resc += 1
            if t == 118:
                # chunk 1 q-transform: small DVE ops here (relayout done by
                # now), big multiply on GpSimd (idle during DP)
                q_transform(1, nc.vector)

        # emit the raw rescale constants + final likelihood; the host takes
        # logs (ACT's Ln table is unreliable across the e+-12 q-space range)
        assert k_resc == 31
        V.tensor_tensor(strip[:, 31:32], alpha[:, 65:66], alpha[:, 130:131], ADD)
        nc.sync.dma_start(loss_ap[:, :], strip[:])


def build_nc():
    nc = bass.Bass("TRN2", target_bir_lowering=False, debug=False)
    # t-chunk-major layout: row ch*BL + b holds y_pred[b, ch*CH:(ch+1)*CH, :]
    yp = nc.dram_tensor("yp16", [NCH * BL, CH, C], f16, kind="ExternalInput").ap()
    e_in = nc.dram_tensor("e_all", [128, BL * NS], f16, kind="ExternalInput").ap()
    m_in = nc.dram_tensor("m_odd", [128, 63], bf16, kind="ExternalInput").ap()
    # t-chunk-major, per chunk [BL, 64*CH]: blank reciprocal replicated x64
    rpb_in = nc.dram_tensor("rpb", [NCH * BL, 64 * CH], bf16, kind="ExternalInput").ap()
    loss = nc.dram_tensor("loss", [BL, 32], f32, kind="ExternalOutput").ap()
    with tile.TileContext(nc) as tc:
        _body(tc, loss, yp, e_in, m_in, rpb_in)
    return nc


def host_label_consts(y_true):
    """Per-core E [c, b*65] (64 label cols + blank, +eps) and skip mask."""
    import ml_dtypes

    lab = np.asarray(y_true).astype(np.int64)  # [B, L]
    outs = []
    for i in range(NCORES):
        lb = lab[i * BL : (i + 1) * BL]  # [128, 64]
        ext = np.concatenate(
            [lb, np.full((BL, 1), C - 1, np.int64)], axis=1
        )  # [128, 65]
        e = (np.arange(128)[:, None, None] == ext[None, :, :]).astype(np.float32)
        e = (e + EPS).astype(np.float16).reshape(128, BL * NS)
        # m[jj] = (lab[jj+1] != lab[jj]): skip allowed from source label jj
        m = (lb[:, 1:] != lb[:, :-1]).astype(ml_dtypes.bfloat16)  # [128, 63]
        outs.append((e, m))
    return outs


_CACHE = {}

# --- BIR legalizer -----------------------------------------------------------
# This container's walrus encodes at most ONE sync wait on SP-queue
# instruction classes (PSEUDO_DMA_DIRECT2D / XPOSE / CTRL): "Too many sync
# wait commands". Tile freely emits >=2 waits per instruction. Split the
# extras onto NoOps inserted just before (same engine stream => semantics
# preserved, waits satisfied in order).
_SPLIT_OPS = {"DMACopy", "DmaTransposeAnt", "DMAGatherAnt", "Drain", "NoOp"}


def _legalize_bir(bir_bytes):
    import orjson

    d = orjson.loads(bir_bytes)
    n_new = 0
    for fn in d.get("functions", []):
        for blk in fn.get("blocks", []):
            insts = blk.get("instructions")
            if not insts:
                continue
            out = []
            for ins in insts:
                si = ins.get("sync_info")
                if si:
                    waits = si.get("on_wait") or []
                    if len(waits) > 1:
                        for w in waits[:-1]:
                            n_new += 1
                            out.append(
                                {
                                    "debug": ins.get("debug", 0),
                                    "engine": ins["engine"],
                                    "ins": [],
                                    "outs": [],
                                    "name": f"ZW-{n_new}",
                                    "opcode": "NoOp",
                                    "sync_info": {"on_wait": [w], "on_update": []},
                                }
                            )
                        si["on_wait"] = [waits[-1]]
                out.append(ins)
            blk["instructions"] = out
    return orjson.dumps(d)


def _install_bir_legalizer():
    import concourse.bass2jax as b2j

    if getattr(b2j, "_ctc_legalizer_installed", False):
        return
    orig = b2j.compile_bir_kernel

    def wrapper(bir_json, tmpdir, neff_name="file.neff"):
        bir_json = _legalize_bir(bir_json)
        return orig(bir_json, tmpdir, neff_name=neff_name)

    b2j.compile_bir_kernel = wrapper
    b2j._ctc_legalizer_installed = True


def make_in_maps(y_true, y_pred):
    import ml_dtypes

    # f16 cast + t-chunk-major reorder: [B, T, C] -> [B, NCH, CH, C] -> per
    # core [NCH, BL, CH, C] flattened to [NCH*BL, CH, C]
    yp16 = np.asarray(y_pred, dtype=np.float16).reshape(B, NCH, CH, C)
    consts = host_label_consts(y_true)
    # blank-prob transforms, replicating the device pipeline exactly:
    # f16 p + eps (f32 matmul) -> bf16 store; reciprocal and log host-side
    pbq = (yp16[:, :, :, C - 1].astype(np.float32) + EPS).astype(
        ml_dtypes.bfloat16
    )  # [B, NCH, CH]
    rpb = (1.0 / pbq.astype(np.float32)).astype(ml_dtypes.bfloat16)
    lnpb = np.log(pbq.astype(np.float64)).sum(axis=(1, 2), dtype=np.float64)
    _CACHE["lnpb"] = lnpb.reshape(B, 1)
    rpb_rep = np.broadcast_to(
        rpb[:, :, None, :], (B, NCH, 64, CH)
    )  # [B, NCH, 64, CH]
    return [
        {
            "yp16": np.ascontiguousarray(
                yp16[i * BL : (i + 1) * BL].transpose(1, 0, 2, 3)
            ).reshape(NCH * BL, CH, C),
            "e_all": consts[i][0],
            "m_odd": consts[i][1],
            "rpb": np.ascontiguousarray(
                rpb_rep[i * BL : (i + 1) * BL].transpose(1, 0, 2, 3)
            ).reshape(NCH * BL, 64 * CH),
        }
        for i in range(NCORES)
    ]


def kernel(y_true, y_pred):
    assert y_pred.shape == (B, T, C) and y_true.shape == (B, L)
    _install_bir_legalizer()
    nc = _CACHE.get("nc")
    if nc is None:
        nc = _CACHE["nc"] = build_nc()
    in_maps = make_in_maps(y_true, y_pred)
    res = run_bass_kernel_spmd(nc, in_maps, list(range(NCORES)))
    strip = np.concatenate(
        [res.results[i]["loss"] for i in range(NCORES)], axis=0
    )  # [B, 32] rescale constants + final likelihood
    ll = np.log(strip.astype(np.float64)).sum(axis=1, keepdims=True)
    return (-(ll + _CACHE["lnpb"])).astype(np.float32)


# revision 23
# speedup vs baseline: 1.1882x; 1.1882x over previous
"""CTC batch loss (Keras convention, blank = C-1) on 8 Trainium2 NeuronCores.

Strategy (pure data parallel, 128 examples per core = 128 SBUF partitions):
  * Prob-domain scaled forward DP, identical math to the reference's
    log-space DP including the log(p + 1e-7) epsilon (folded into the
    gather as E = onehot + eps; softmax rows sum to 1 so the gathered
    value is exactly p + eps).
  * Gather via per-example one-hot matmul on the TensorEngine: y_pred
    arrives host-cast to f16 in t-chunk-major layout; XBAR dma-transpose
    (t,c)->(c,t); matmul E^T . X^T -> PSUM f32 [65, t]; ACT-engine escape
    copy casting to bf16; DMA re-layout into batch-partitioned p_store.
  * Serial DP over T in bf16 with a split even/odd contiguous layout so
    most hot ops run in the DVE 2x/4x packed perf modes: 6 DVE ops per
    step, no cross-engine hops; rescale every 8 steps, log-corrections
    collected in a strip and reduced once at the end.
"""

import sys
from contextlib import ExitStack

import numpy as np

for _p in ("/opt/trn_rl_repo",):
    if _p not in sys.path:
        sys.path.insert(0, _p)

import concourse.bass as bass
import concourse.tile as tile
from concourse import mybir
from concourse.bass_utils import run_bass_kernel_spmd

# Problem constants (hardcoded per spec nn_CTC_55808805045003)
B, T, C, L = 1024, 256, 128, 64
NCORES = 8
BL = B // NCORES          # 128 examples per core
S = 2 * L + 1             # 129 extended labels
NS = L + 1                # 65 gather columns (64 labels + blank)
EPS = 1e-7
CH = 128                  # time chunk
NCH = T // CH             # 2
RESC = 8                  # rescale period
GT = 8                    # examples per transpose/matmul-group/relayout DMA

f32 = mybir.dt.float32
bf16 = mybir.dt.bfloat16
f16 = mybir.dt.float16

ADD = mybir.AluOpType.add
MULT = mybir.AluOpType.mult
AX_X = mybir.AxisListType.X
AFT = mybir.ActivationFunctionType


def _body(tc, loss_ap, yp16, e_ap, m_ap):
    nc = tc.nc
    with ExitStack() as ctx:
        const = ctx.enter_context(tc.tile_pool(name="const", bufs=1))
        xtp = ctx.enter_context(tc.tile_pool(name="xt", bufs=6))
        gps = ctx.enter_context(tc.tile_pool(name="gpsum", bufs=3, space="PSUM"))
        gsb = ctx.enter_context(tc.tile_pool(name="gsb", bufs=6))
        tiny = ctx.enter_context(tc.tile_pool(name="tiny", bufs=8))

        # ---- label-derived constants (host-computed, DMA'd in) ----
        Eall = const.tile([128, BL * NS], f16)
        nc.sync.dma_start(Eall[:], e_ap[:, :])
        m_odd = const.tile([128, 63], bf16)
        nc.sync.dma_start(m_odd[:], m_ap[:, :])

        # per-chunk gathered probs (bf16): p_store[ch][b, s*CH + t]
        # rows 0..63 are divided by the blank row in place (q-transform)
        p_stores = [
            const.tile([128, NS * CH], bf16, name=f"p_store{ch}") for ch in range(NCH)
        ]
        ps3 = [
            p_stores[ch][:].rearrange("p (s t) -> p s t", s=NS) for ch in range(NCH)
        ]
        # strip: 31 rescale logs + 1 final lik col
        strip = const.tile([128, 32], f32)

        def gather_chunk(ch):
            for g in range(BL // GT):
                b0 = g * GT
                r0 = ch * BL + b0
                eng_t = nc.sync if (g % 2 == 0) else nc.scalar
                # one XBAR transpose covers GT examples: [GT*CH, C] -> [C, GT*CH]
                xt = xtp.tile([C, GT * CH], f16)
                eng_t.dma_start_transpose(
                    xt[:],
                    yp16[r0 : r0 + GT, :, :].rearrange("b t c -> (b t) c"),
                )
                gp = gps.tile([NS, GT * CH], f32)
                for i in range(GT):
                    b = b0 + i
                    # out[s, t] = sum_c E[c, s] * xt[c, t]
                    nc.tensor.matmul(
                        gp[:, i * CH : (i + 1) * CH],
                        Eall[:, b * NS : (b + 1) * NS],
                        xt[:, i * CH : (i + 1) * CH],
                        start=True,
                        stop=True,
                    )
                # escape PSUM f32 -> SBUF bf16 (GpSimd can't read PSUM):
                # DVE pre-DP for chunk 0, ACT during DP for chunk 1
                gs = gsb.tile([NS, GT * CH], bf16)
                if ch == 0:
                    nc.vector.tensor_copy(gs[:], gp[:])
                else:
                    nc.scalar.copy(gs[:], gp[:])
                # per-example re-layout into batch partitions; chunk 0 uses
                # three HWDGE queues (DVE's is free pre-DP), chunk 1 two
                for i in range(GT):
                    b = b0 + i
                    if ch == 0:
                        r = b % 16
                        eng = nc.sync if r < 5 else (nc.scalar if r < 10 else nc.gpsimd)
                    else:
                        eng = nc.scalar if (b % 2) else nc.sync
                    eng.dma_start(
                        ps3[ch][b : b + 1, :, :], gs[:, i * CH : (i + 1) * CH]
                    )

        gather_chunk(0)
        pbl0 = tiny.tile([128, CH], f32)
        nc.vector.tensor_copy(pbl0[:], ps3[0][:, NS - 1 : NS, :].squeeze(1))
        gather_chunk(1)
        pbl = [pbl0, None]

        # ---- DP state (bf16, split even/odd contiguous layout) ----
        # alpha cols: 0 = zero guard; 1..65 = even states i=0..64 (s=2i);
        # 66 = zero guard; 67..130 = odd states j=0..63 (s=2j+1); 131 pad
        alpha = const.tile([128, 132], bf16)
        AM = const.tile([128, 64], bf16)   # AM[k] = mask*alpha_odd[k-1]; AM[0]=0
        uo = const.tile([128, 64], bf16)
        vo = const.tile([128, 64], bf16)
        r_ap = tiny.tile([128, 1], f32)

        nc.vector.memset(alpha[:], 0.0)
        nc.vector.memset(AM[:], 0.0)

        # t = 0 init: alpha[s=0] = p_bl(0), alpha[s=1] = p_lab0(0),
        # AM[1] = m0 * alpha_odd0
        nc.vector.tensor_copy(alpha[:, 1:2], ps3[0][:, NS - 1 : NS, 0:1].squeeze(2))
        nc.vector.tensor_copy(alpha[:, 67:68], ps3[0][:, 0:1, 0:1].squeeze(2))
        nc.vector.tensor_tensor(AM[:, 1:2], alpha[:, 67:68], m_odd[:, 0:1], MULT)

        V = nc.vector
        k_resc = 0
        for t in range(1, T):
            ch, tt = divmod(t, CH)
            ps = ps3[ch]
            pl = ps[:, 0:64, tt : tt + 1].squeeze(2)  # [128, 64] label probs
            # uo[j] = alpha_odd[j] + alpha_even[j]
            V.tensor_tensor(uo[:], alpha[:, 67:131], alpha[:, 1:65], ADD)
            # vo[j] = uo[j] + AM[j]
            V.tensor_tensor(vo[:], uo[:], AM[:], ADD)
            # even' = even + odd shifted (no multiply in q-space)
            V.tensor_tensor(alpha[:, 1:66], alpha[:, 1:66], alpha[:, 66:131], ADD)
            # odd' = vo * p_lab
            V.tensor_tensor(alpha[:, 67:131], vo[:], pl, MULT)
            # even' *= p_blank (per-partition f32 scalar -> 4x mode)
            V.tensor_scalar(alpha[:, 1:66], alpha[:, 1:66], pbl[ch][:, tt : tt + 1], None, MULT)
            # AM' for next step: mask * alpha_odd' shifted
            V.tensor_tensor(AM[:, 1:64], alpha[:, 67:130], m_odd[:], MULT)
            if t % RESC == 0 and t < T - RESC + 1:
                V.tensor_reduce(strip[:, k_resc : k_resc + 1], alpha[:, 1:131], AX_X, ADD)
                V.reciprocal(r_ap[:], strip[:, k_resc : k_resc + 1])
                V.tensor_scalar(alpha[:, 1:131], alpha[:, 1:131], r_ap[:], None, MULT)
                V.tensor_scalar(AM[:], AM[:], r_ap[:], None, MULT)
                k_resc += 1
            if t == 118:
                # chunk 1 blank row -> f32 (relayout is done by now)
                pbl[1] = tiny.tile([128, CH], f32, name="pbl1")
                nc.vector.tensor_copy(pbl[1][:], ps3[1][:, NS - 1 : NS, :].squeeze(1))

        # emit the raw rescale constants + final likelihood; the host takes
        # logs (ACT's Ln table is unreliable across the e+-12 q-space range)
        assert k_resc == 31
        V.tensor_tensor(strip[:, 31:32], alpha[:, 65:66], alpha[:, 130:131], ADD)
        nc.sync.dma_start(loss_ap[:, :], strip[:])


def build_nc():
    nc = bass.Bass("TRN2", target_bir_lowering=False, debug=False)
    # t-chunk-major layout: row ch*BL + b holds y_pred[b, ch*CH:(ch+1)*CH, :]
    yp = nc.dram_tensor("yp16", [NCH * BL, CH, C], f16, kind="ExternalInput").ap()
    e_in = nc.dram_tensor("e_all", [128, BL * NS], f16, kind="ExternalInput").ap()
    m_in = nc.dram_tensor("m_odd", [128, 63], bf16, kind="ExternalInput").ap()
    loss = nc.dram_tensor("loss", [BL, 32], f32, kind="ExternalOutput").ap()
    with tile.TileContext(nc) as tc:
        _body(tc, loss, yp, e_in, m_in)
    return nc


def host_label_consts(y_true):
    """Per-core E [c, b*65] (64 label cols + blank, +eps) and skip mask."""
    import ml_dtypes

    lab = np.asarray(y_true).astype(np.int64)  # [B, L]
    outs = []
    for i in range(NCORES):
        lb = lab[i * BL : (i + 1) * BL]  # [128, 64]
        ext = np.concatenate(
            [lb, np.full((BL, 1), C - 1, np.int64)], axis=1
        )  # [128, 65]
        e = (np.arange(128)[:, None, None] == ext[None, :, :]).astype(np.float32)
        e = (e + EPS).astype(np.float16).reshape(128, BL * NS)
        # m[jj] = (lab[jj+1] != lab[jj]): skip allowed from source label jj
        m = (lb[:, 1:] != lb[:, :-1]).astype(ml_dtypes.bfloat16)  # [128, 63]
        outs.append((e, m))
    return outs


_CACHE = {}

# --- BIR legalizer -----------------------------------------------------------
# This container's walrus encodes at most ONE sync wait on SP-queue
# instruction classes (PSEUDO_DMA_DIRECT2D / XPOSE / CTRL): "Too many sync
# wait commands". Tile freely emits >=2 waits per instruction. Split the
# extras onto NoOps inserted just before (same engine stream => semantics
# preserved, waits satisfied in order).
_SPLIT_OPS = {"DMACopy", "DmaTransposeAnt", "DMAGatherAnt", "Drain", "NoOp"}


def _legalize_bir(bir_bytes):
    import orjson

    d = orjson.loads(bir_bytes)
    n_new = 0
    for fn in d.get("functions", []):
        for blk in fn.get("blocks", []):
            insts = blk.get("instructions")
            if not insts:
                continue
            out = []
            for ins in insts:
                si = ins.get("sync_info")
                if si:
                    waits = si.get("on_wait") or []
                    if len(waits) > 1:
                        for w in waits[:-1]:
                            n_new += 1
                            out.append(
                                {
                                    "debug": ins.get("debug", 0),
                                    "engine": ins["engine"],
                                    "ins": [],
                                    "outs": [],
                                    "name": f"ZW-{n_new}",
                                    "opcode": "NoOp",
                                    "sync_info": {"on_wait": [w], "on_update": []},
                                }
                            )
                        si["on_wait"] = [waits[-1]]
                out.append(ins)
            blk["instructions"] = out
    return orjson.dumps(d)


def _install_bir_legalizer():
    import concourse.bass2jax as b2j

    if getattr(b2j, "_ctc_legalizer_installed", False):
        return
    orig = b2j.compile_bir_kernel

    def wrapper(bir_json, tmpdir, neff_name="file.neff"):
        bir_json = _legalize_bir(bir_json)
        return orig(bir_json, tmpdir, neff_name=neff_name)

    b2j.compile_bir_kernel = wrapper
    b2j._ctc_legalizer_installed = True


def make_in_maps(y_true, y_pred):
    # f16 cast + t-chunk-major reorder: [B, T, C] -> [B, NCH, CH, C] -> per
    # core [NCH, BL, CH, C] flattened to [NCH*BL, CH, C]
    yp16 = np.asarray(y_pred, dtype=np.float16).reshape(B, NCH, CH, C)
    consts = host_label_consts(y_true)
    return [
        {
            "yp16": np.ascontiguousarray(
                yp16[i * BL : (i + 1) * BL].transpose(1, 0, 2, 3)
            ).reshape(NCH * BL, CH, C),
            "e_all": consts[i][0],
            "m_odd": consts[i][1],
        }
        for i in range(NCORES)
    ]


def kernel(y_true, y_pred):
    assert y_pred.shape == (B, T, C) and y_true.shape == (B, L)
    _install_bir_legalizer()
    nc = _CACHE.get("nc")
    if nc is None:
        nc = _CACHE["nc"] = build_nc()
    in_maps = make_in_maps(y_true, y_pred)
    res = run_bass_kernel_spmd(nc, in_maps, list(range(NCORES)))
    strip = np.concatenate(
        [res.results[i]["loss"] for i in range(NCORES)], axis=0
    )  # [B, 32] rescale constants + final likelihood
    ll = np.log(strip.astype(np.float64)).sum(axis=1, keepdims=True)
    return (-ll).astype(np.float32)
